# revision 2
# baseline (speedup 1.0000x reference)
"""Trainium2 Bass kernel for nn_DenseStationaryQMatrixDecoder.

Reference math: Q = rownorm(exp(logQ) * (1-I)) - I  (a 4x4 CTMC rate matrix),
output = broadcast(row0(expm(Q*1000)), (V, S, A)).  expm(Q*1000) converges to
the rank-1 stationary matrix 1*pi^T, so every output element is pi[a].

Device strategy (per core, 8 cores data-parallel over V):
  1. Compute P = rownorm(exp(logQ)*(1-I)) on-chip.  The host packs logQ
     with -100 added on the diagonal, so exp() zeroes the diagonal with
     no extra mask op.  Row sums + reciprocal + scale run back-to-back
     on DVE (no ACT accumulator read, one fewer cross-engine hop).
  2. Converge by repeated squaring: P^(2^NSQ) -> all rows == pi.  Squaring
     without transposes: keep (X, X^T); X2 = matmul(lhsT=X^T, rhs=X),
     X2^T = matmul(lhsT=X, rhs=X^T).  NSQ=3 -> P^8 (|lambda2(P)| ~ 0.38,
     so P^8 is within ~4e-4 of pi; gate is 2e-2).  All matmul operands are
     bf16 (fp32 PE matmuls run 2 passes; bf16 runs 1 -- host-emulated
     max rel err of the full bf16 chain is 2.6e-3, still 7x under the
     gate).  The bf16 identity for the first transpose is packed into the
     f32 input blob and read back via AP bitcast.  P^T comes from one
     matmul against it (lhsT=P -> P^T), off the activation engine.
  3. The final squaring is fused with the partition broadcast:
     row0(X@X) = (XT[:,0])^T @ X, so matmul(lhsT=XT[:,0] bcast to (4,128),
     rhs=X) yields a (128, 4) PSUM tile whose every row is pi.
  4. Output streaming (16 SDMA engines at their ~27 GB/s ceiling; packet
     cost measured on HW: 4KiB pkt ~164 ns, 16KiB pkt ~612 ns, so bigger
     descriptors win once the pattern is wide enough):
       - Copy pi into a tiny [128, 4] SBUF seed, fill a 4 KiB-per-partition
         narrow slice (DVE + idle ACT split), and immediately stream the
         first 1 MiB from it (4 KiB descriptors).
       - Widen the pattern to 12 KiB per partition (DVE), then stream
         6 MiB as one dma_start (12 KiB descriptors) and the last 1 MiB
         as a second (8 KiB descriptors), so the SDMA queues never drain
         and the small-descriptor region is only the first 1 MiB.
     The DRAM output is declared flat [2097152]; the bytes are one value
     broadcast, so any host reshape is valid.
  5. Residual variance: under cross-core HBM load one SDMA engine
     (usually 15) intermittently drops to ~60% rate for a few us and
     finishes ~2-3 us after the rest.  Per-engine byte skew was tried in
     a previous session and refuted: partition-sliced DMAs are re-split
     across engines by the AP normalizer's own policy, so a straggling
     engine cannot be de-loaded.  Accepted.
"""

import sys

if "/opt/trn_rl_repo" not in sys.path:
    sys.path.insert(0, "/opt/trn_rl_repo")

import numpy as np

A = 4
V = 512
S = 8192
N_CORES = 8
PER_CORE = V * S * A // N_CORES  # 2,097,152 f32 = 8 MiB
P128 = 128
FREE = 3072                      # full pattern width (12 KiB per partition)
W0 = 1024                        # first-chunk pattern width (4 KiB/partition)
WC = 2048                        # last-chunk descriptor width (8 KiB)
NSQ = 3                          # total squarings incl. the fused final one

_cache = {}


def _build():
    import concourse.bacc as bacc
    import concourse.mybir as mybir
    import concourse.tile as tile

    f32 = mybir.dt.float32
    bf16 = mybir.dt.bfloat16
    AF = mybir.ActivationFunctionType
    AX = mybir.AxisListType

    nc = bacc.Bacc(
        "TRN2", target_bir_lowering=False, debug=False, num_devices=N_CORES
    )
    blob = nc.dram_tensor("blob", [A, 2 * A], f32, kind="ExternalInput").ap()
    out = nc.dram_tensor("out", [PER_CORE], f32, kind="ExternalOutput").ap()

    with tile.TileContext(nc) as tc:
        with (
            tc.tile_pool(name="small", bufs=1) as sp,
            tc.tile_pool(name="loop", bufs=3) as lp,
            tc.tile_pool(name="patt", bufs=1) as pp,
            tc.tile_pool(name="ps1", bufs=1, space="PSUM") as ps1,
            tc.tile_pool(name="ps2", bufs=3, space="PSUM") as ps2,
        ):
            bt = sp.tile([A, 2 * A], f32)
            nc.sync.dma_start(out=bt[:], in_=blob, single_packet=True)
            lq = bt[:, 0:A]                 # logq, diagonal pre-masked to -100
            # cols [A : A+2] hold the bf16 identity's raw bits
            eye_bf = bt[:, A : A + 2].bitcast(bf16)   # [A, A] bf16

            E = sp.tile([A, A], f32)        # exp(lq): zero diagonal
            nc.scalar.activation(out=E[:], in_=lq, func=AF.Exp)
            s = sp.tile([A, 1], f32)
            nc.vector.reduce_sum(out=s[:], in_=E[:], axis=AX.X)
            r = sp.tile([A, 1], f32)
            nc.vector.reciprocal(out=r[:], in_=s[:])

            # X = P = diag(r) @ E, cast to bf16 for 1-pass PE matmuls
            X0 = sp.tile([A, A], bf16)
            nc.vector.tensor_scalar_mul(out=X0[:], in0=E[:], scalar1=r[:])
            # X^T via one matmul: lhsT=X0 -> X0^T @ I = P^T  (no PE transpose)
            pt = ps1.tile([A, A], f32)
            nc.tensor.matmul(pt[:], lhsT=X0[:], rhs=eye_bf, start=True, stop=True)
            XT0 = sp.tile([A, A], bf16)
            nc.vector.tensor_copy(out=XT0[:], in_=pt[:])

            # Squaring loop.  Both matmuls of an iteration write bank-aligned
            # quads of ONE two-bank PSUM tile, so a single strided DVE copy
            # (instead of two engine-split copies) pulls X2 and X2^T back to
            # SBUF side by side.
            BANK = 512  # f32 elems per PSUM bank row
            Xa, XTa = X0, XT0
            for _ in range(NSQ - 1):
                pr = ps2.tile([A, 2 * BANK], f32)
                nc.tensor.matmul(
                    pr[:, 0:A], lhsT=XTa[:], rhs=Xa[:], start=True, stop=True
                )
                nc.tensor.matmul(
                    pr[:, BANK : BANK + A], lhsT=Xa[:], rhs=XTa[:],
                    start=True, stop=True,
                )
                pair = lp.tile([A, 2 * A], bf16)
                psrc = pr[:].rearrange("p (b f) -> p b f", b=2)[:, :, 0:A]
                pdst = pair[:].rearrange("p (b f) -> p b f", b=2)
                nc.vector.tensor_copy(out=pdst, in_=psrc)
                Xa, XTa = pair[:, 0:A], pair[:, A : 2 * A]

            # Fused last squaring + broadcast:
            # row0(X@X) = (XT[:,0])^T @ X, replicated to 128 partitions by
            # free-dim-broadcasting the stationary operand.
            pbig = ps1.tile([P128, A], f32)
            nc.tensor.matmul(
                pbig[:],
                lhsT=XTa[:, 0:1].to_broadcast((A, P128)),
                rhs=Xa[:],
                start=True,
                stop=True,
            )

            # Stream out: narrow fill -> first chunk ASAP -> widen -> rest.
            # DVE broadcast-reads from PSUM run at ~1.2 ns/elem vs ~0.6 from
            # SBUF (measured), so hop pi through a tiny SBUF seed first.
            seed = sp.tile([P128, A], f32)
            nc.vector.tensor_copy(out=seed[:], in_=pbig[:])
            patt = pp.tile([P128, FREE], f32)
            # First-chunk fill split across DVE and the idle activation
            # engine (its Copy/scale funcs share the Exp table set, so no
            # extra ACT_TABLE_LOAD) to shave the critical path.
            WD = 768  # DVE's share; ACT fills the rest of W0
            p3a = patt[:, 0:WD].rearrange("p (r a) -> p r a", a=A)
            src_a = seed[:].unsqueeze(1).to_broadcast((P128, WD // A, A))
            nc.vector.tensor_copy(out=p3a, in_=src_a)
            p3c = patt[:, WD:W0].rearrange("p (r a) -> p r a", a=A)
            src_c = seed[:].unsqueeze(1).to_broadcast((P128, (W0 - WD) // A, A))
            nc.scalar.mul(p3c, src_c, 1.0)
            # A: first 1 MiB from the narrow slice, all 128 partitions.
            o0, l0 = 0, P128 * 2 * W0
            nc.sync.dma_start(
                out=out[o0 : o0 + l0].rearrange(
                    "(p c f) -> p c f", p=P128, f=W0
                ),
                in_=patt[:, 0:W0].unsqueeze(1).to_broadcast((P128, 2, W0)),
            )
            p3b = patt[:, W0:FREE].rearrange("p (r a) -> p r a", a=A)
            src_b = seed[:].unsqueeze(1).to_broadcast(
                (P128, (FREE - W0) // A, A)
            )
            nc.vector.tensor_copy(out=p3b, in_=src_b)
            # B: middle 6 MiB as one dma_start, 12 KiB descriptors.
            o1, l1 = o0 + l0, P128 * 4 * FREE
            nc.sync.dma_start(
                out=out[o1 : o1 + l1].rearrange(
                    "(p c f) -> p c f", p=P128, f=FREE
                ),
                in_=patt[:].unsqueeze(1).to_broadcast((P128, 4, FREE)),
            )
            # C: last 1 MiB, 8 KiB descriptors.
            o2, l2 = o1 + l1, P128 * WC
            nc.sync.dma_start(
                out=out[o2 : o2 + l2].rearrange("(p f) -> p f", p=P128),
                in_=patt[:, 0:WC],
            )
            assert o2 + l2 == PER_CORE

    nc.compile()
    return nc


def _get_nc():
    if "nc" not in _cache:
        _cache["nc"] = _build()
    return _cache["nc"]


def _in_map(log_Q_matrix_AxA):
    logq = np.asarray(log_Q_matrix_AxA, dtype=np.float32).reshape(A, A)
    eye = np.eye(A, dtype=np.float32)
    blob = np.zeros((A, 2 * A), dtype=np.float32)
    blob[:, 0:A] = logq - 100.0 * eye
    # bf16 identity bits: 1.0 = 0x3F80, packed pairs into f32 lanes
    eye_bits = np.where(np.eye(A, dtype=bool), 0x3F80, 0).astype("<u2")
    blob[:, A : A + 2] = eye_bits.view("<u4").view("<f4")
    return {"blob": np.ascontiguousarray(blob)}


def kernel(
    embeddings_VxD=None, site_positions_SxC=None, log_Q_matrix_AxA=None, **_unused
):
    from concourse.bass_utils import run_bass_kernel_spmd

    nc = _get_nc()
    im = _in_map(log_Q_matrix_AxA)
    res = run_bass_kernel_spmd(
        nc, [dict(im) for _ in range(N_CORES)], core_ids=list(range(N_CORES))
    )
    parts = [r["out"].reshape(V // N_CORES, S, A) for r in res.results]
    return np.concatenate(parts, axis=0)


# revision 5
# speedup vs baseline: 1.0015x; 1.0015x over previous
"""Trainium2 Bass kernel for nn_DenseStationaryQMatrixDecoder.

Reference math: Q = rownorm(exp(logQ) * (1-I)) - I  (a 4x4 CTMC rate matrix),
output = broadcast(row0(expm(Q*1000)), (V, S, A)).  expm(Q*1000) converges to
the rank-1 stationary matrix 1*pi^T, so every output element is pi[a].

Device strategy (per core, 8 cores data-parallel over V):
  1. Compute P = rownorm(exp(logQ)*(1-I)) on-chip.  The host packs logQ
     with -100 added on the diagonal, so exp() zeroes the diagonal with
     no extra mask op.  Row sums + reciprocal + scale run back-to-back
     on DVE (no ACT accumulator read, one fewer cross-engine hop).
  2. Converge by repeated squaring: P^(2^NSQ) -> all rows == pi.  Squaring
     without transposes: keep (X, X^T); X2 = matmul(lhsT=X^T, rhs=X),
     X2^T = matmul(lhsT=X, rhs=X^T).  NSQ=3 -> P^8 (|lambda2(P)| ~ 0.38,
     so P^8 is within ~4e-4 of pi; gate is 2e-2).  All matmul operands are
     bf16 (fp32 PE matmuls run 2 passes; bf16 runs 1 -- host-emulated
     max rel err of the full bf16 chain is 2.6e-3, still 7x under the
     gate).  The bf16 identity for the first transpose is packed into the
     f32 input blob and read back via AP bitcast.  P^T comes from one
     matmul against it (lhsT=P -> P^T), off the activation engine.
  3. The final squaring is fused with the partition broadcast:
     row0(X@X) = (XT[:,0])^T @ X, so matmul(lhsT=XT[:,0] bcast to (4,128),
     rhs=X) yields a (128, 4) PSUM tile whose every row is pi.
  4. Output streaming (16 SDMA engines at their ~27 GB/s ceiling; packet
     cost measured on HW: 4KiB pkt ~164 ns, 16KiB pkt ~612 ns, so bigger
     descriptors win once the pattern is wide enough):
       - Copy pi into a tiny [128, 4] SBUF seed, fill a 4 KiB-per-partition
         narrow slice (DVE + idle ACT split), and immediately stream the
         first 1 MiB from it (4 KiB descriptors).
       - Widen the pattern to 12 KiB per partition (DVE), then stream
         6 MiB as one dma_start (12 KiB descriptors) and the last 1 MiB
         as a second (8 KiB descriptors), so the SDMA queues never drain
         and the small-descriptor region is only the first 1 MiB.
     The DRAM output is declared flat [2097152]; the bytes are one value
     broadcast, so any host reshape is valid.
  5. Residual variance: under cross-core HBM load one SDMA engine
     (usually 15) intermittently drops to ~60% rate for a few us and
     finishes ~2-3 us after the rest.  Per-engine byte skew was tried in
     a previous session and refuted: partition-sliced DMAs are re-split
     across engines by the AP normalizer's own policy, so a straggling
     engine cannot be de-loaded.  Accepted.
"""

import sys

if "/opt/trn_rl_repo" not in sys.path:
    sys.path.insert(0, "/opt/trn_rl_repo")

import numpy as np

A = 4
V = 512
S = 8192
N_CORES = 8
PER_CORE = V * S * A // N_CORES  # 2,097,152 f32 = 8 MiB
P128 = 128
FREE = 3072                      # full pattern width (12 KiB per partition)
W0 = 1024                        # first-chunk pattern width (4 KiB/partition)
WC = 2048                        # last-chunk descriptor width (8 KiB)
NSQ = 3                          # total squarings incl. the fused final one

_cache = {}


def _build():
    import concourse.bacc as bacc
    import concourse.mybir as mybir
    import concourse.tile as tile

    f32 = mybir.dt.float32
    bf16 = mybir.dt.bfloat16
    AF = mybir.ActivationFunctionType

    nc = bacc.Bacc(
        "TRN2", target_bir_lowering=False, debug=False, num_devices=N_CORES
    )
    blob = nc.dram_tensor("blob", [A, 2 * A], f32, kind="ExternalInput").ap()
    out = nc.dram_tensor("out", [PER_CORE], f32, kind="ExternalOutput").ap()

    with tile.TileContext(nc) as tc:
        with (
            tc.tile_pool(name="small", bufs=1) as sp,
            tc.tile_pool(name="loop", bufs=3) as lp,
            tc.tile_pool(name="patt", bufs=1) as pp,
            tc.tile_pool(name="ps1", bufs=1, space="PSUM") as ps1,
            tc.tile_pool(name="ps2", bufs=3, space="PSUM") as ps2,
        ):
            bt = sp.tile([A, 2 * A], f32)
            nc.sync.dma_start(out=bt[:], in_=blob, single_packet=True)
            lq = bt[:, 0:A]                 # logq, diagonal pre-masked to -100
            # cols [A : A+2] hold the bf16 identity's raw bits
            eye_bf = bt[:, A : A + 2].bitcast(bf16)   # [A, A] bf16

            E = sp.tile([A, A], f32)        # exp(lq): zero diagonal
            s = sp.tile([A, 1], f32)        # fused row sums
            nc.scalar.activation(out=E[:], in_=lq, func=AF.Exp, accum_out=s[:])
            r = sp.tile([A, 1], f32)
            nc.vector.reciprocal(out=r[:], in_=s[:])

            # X = P = diag(r) @ E, cast to bf16 for 1-pass PE matmuls
            X0 = sp.tile([A, A], bf16)
            nc.vector.tensor_scalar_mul(out=X0[:], in0=E[:], scalar1=r[:])
            # X^T via one matmul: lhsT=X0 -> X0^T @ I = P^T  (no PE transpose)
            pt = ps1.tile([A, A], f32)
            nc.tensor.matmul(pt[:], lhsT=X0[:], rhs=eye_bf, start=True, stop=True)
            XT0 = sp.tile([A, A], bf16)
            nc.vector.tensor_copy(out=XT0[:], in_=pt[:])

            # Squaring loop.  Both matmuls of an iteration write bank-aligned
            # quads of ONE two-bank PSUM tile, so a single strided DVE copy
            # (instead of two engine-split copies) pulls X2 and X2^T back to
            # SBUF side by side.
            BANK = 512  # f32 elems per PSUM bank row
            Xa, XTa = X0, XT0
            for _ in range(NSQ - 1):
                pr = ps2.tile([A, 2 * BANK], f32)
                nc.tensor.matmul(
                    pr[:, 0:A], lhsT=XTa[:], rhs=Xa[:], start=True, stop=True
                )
                nc.tensor.matmul(
                    pr[:, BANK : BANK + A], lhsT=Xa[:], rhs=XTa[:],
                    start=True, stop=True,
                )
                pair = lp.tile([A, 2 * A], bf16)
                psrc = pr[:].rearrange("p (b f) -> p b f", b=2)[:, :, 0:A]
                pdst = pair[:].rearrange("p (b f) -> p b f", b=2)
                nc.vector.tensor_copy(out=pdst, in_=psrc)
                Xa, XTa = pair[:, 0:A], pair[:, A : 2 * A]

            # Fused last squaring + broadcast:
            # row0(X@X) = (XT[:,0])^T @ X, replicated to 128 partitions by
            # free-dim-broadcasting the stationary operand.
            pbig = ps1.tile([P128, A], f32)
            nc.tensor.matmul(
                pbig[:],
                lhsT=XTa[:, 0:1].to_broadcast((A, P128)),
                rhs=Xa[:],
                start=True,
                stop=True,
            )

            # Stream out: narrow fill -> first chunk ASAP -> widen -> rest.
            # DVE broadcast-reads from PSUM run at ~1.2 ns/elem vs ~0.6 from
            # SBUF (measured), so hop pi through a tiny SBUF seed first.
            seed = sp.tile([P128, A], f32)
            nc.vector.tensor_copy(out=seed[:], in_=pbig[:])
            patt = pp.tile([P128, FREE], f32)
            # First-chunk fill split across DVE and the idle activation
            # engine (its Copy/scale funcs share the Exp table set, so no
            # extra ACT_TABLE_LOAD) to shave the critical path.
            WD = 768  # DVE's share; ACT fills the rest of W0
            p3a = patt[:, 0:WD].rearrange("p (r a) -> p r a", a=A)
            src_a = seed[:].unsqueeze(1).to_broadcast((P128, WD // A, A))
            nc.vector.tensor_copy(out=p3a, in_=src_a)
            p3c = patt[:, WD:W0].rearrange("p (r a) -> p r a", a=A)
            src_c = seed[:].unsqueeze(1).to_broadcast((P128, (W0 - WD) // A, A))
            nc.scalar.mul(p3c, src_c, 1.0)
            # A: first 1 MiB from the narrow slice, all 128 partitions.
            o0, l0 = 0, P128 * 2 * W0
            nc.sync.dma_start(
                out=out[o0 : o0 + l0].rearrange(
                    "(p c f) -> p c f", p=P128, f=W0
                ),
                in_=patt[:, 0:W0].unsqueeze(1).to_broadcast((P128, 2, W0)),
            )
            # D1-D3: 1 MiB of top-up via SWDGE over partitions that exclude
            # SDMA engine 15's port set {92-95, 124-127}.  SWDGE's
            # CounterMachine assigns descriptors to engines by source
            # partition (unlike HWDGE's AP-normalizer re-spray), so engine
            # 15 -- the chronically interfered-with engine -- ends up with
            # ~12% fewer bytes and stops being the stream's critical path.
            # These read only patt[:, 0:W0], so they are gated on the
            # narrow fill and issue early; Q7 emission latency hides under
            # chunk A's drain.
            od1, ld1 = o0 + l0, 92 * 2 * W0
            nc.gpsimd.dma_start(
                out=out[od1 : od1 + ld1].rearrange(
                    "(p c f) -> p c f", p=92, f=W0
                ),
                in_=patt[0:92, 0:W0].unsqueeze(1).to_broadcast((92, 2, W0)),
            )
            od2, ld2 = od1 + ld1, 28 * 2 * W0
            nc.gpsimd.dma_start(
                out=out[od2 : od2 + ld2].rearrange(
                    "(p c f) -> p c f", p=28, f=W0
                ),
                in_=patt[96:124, 0:W0].unsqueeze(1).to_broadcast((28, 2, W0)),
            )
            od3, ld3 = od2 + ld2, 16 * W0
            nc.gpsimd.dma_start(
                out=out[od3 : od3 + ld3].rearrange("(p f) -> p f", p=16),
                in_=patt[0:16, 0:W0],
            )
            p3b = patt[:, W0:FREE].rearrange("p (r a) -> p r a", a=A)
            src_b = seed[:].unsqueeze(1).to_broadcast(
                (P128, (FREE - W0) // A, A)
            )
            nc.vector.tensor_copy(out=p3b, in_=src_b)
            # B: remaining 6 MiB as one dma_start, 12 KiB descriptors.
            o1, l1 = od3 + ld3, P128 * 4 * FREE
            nc.sync.dma_start(
                out=out[o1 : o1 + l1].rearrange(
                    "(p c f) -> p c f", p=P128, f=FREE
                ),
                in_=patt[:].unsqueeze(1).to_broadcast((P128, 4, FREE)),
            )
            assert o1 + l1 == PER_CORE

    nc.compile()
    return nc


def _get_nc():
    if "nc" not in _cache:
        _cache["nc"] = _build()
    return _cache["nc"]


def _in_map(log_Q_matrix_AxA):
    logq = np.asarray(log_Q_matrix_AxA, dtype=np.float32).reshape(A, A)
    eye = np.eye(A, dtype=np.float32)
    blob = np.zeros((A, 2 * A), dtype=np.float32)
    blob[:, 0:A] = logq - 100.0 * eye
    # bf16 identity bits: 1.0 = 0x3F80, packed pairs into f32 lanes
    eye_bits = np.where(np.eye(A, dtype=bool), 0x3F80, 0).astype("<u2")
    blob[:, A : A + 2] = eye_bits.view("<u4").view("<f4")
    return {"blob": np.ascontiguousarray(blob)}


def kernel(
    embeddings_VxD=None, site_positions_SxC=None, log_Q_matrix_AxA=None, **_unused
):
    from concourse.bass_utils import run_bass_kernel_spmd

    nc = _get_nc()
    im = _in_map(log_Q_matrix_AxA)
    res = run_bass_kernel_spmd(
        nc, [dict(im) for _ in range(N_CORES)], core_ids=list(range(N_CORES))
    )
    parts = [r["out"].reshape(V // N_CORES, S, A) for r in res.results]
    return np.concatenate(parts, axis=0)


# revision 6
# speedup vs baseline: 1.0879x; 1.0863x over previous
"""Trainium2 Bass kernel for nn_DenseStationaryQMatrixDecoder.

Reference math: Q = rownorm(exp(logQ) * (1-I)) - I  (a 4x4 CTMC rate matrix),
output = broadcast(row0(expm(Q*1000)), (V, S, A)).  expm(Q*1000) converges to
the rank-1 stationary matrix 1*pi^T, so every output element is pi[a].

Device strategy (per core, 8 cores data-parallel over V):
  1. Compute P = rownorm(exp(logQ)*(1-I)) on-chip.  The host packs logQ
     with -100 added on the diagonal, so exp() zeroes the diagonal with
     no extra mask op.  Row sums + reciprocal + scale run back-to-back
     on DVE (no ACT accumulator read, one fewer cross-engine hop).
  2. Converge by repeated squaring: P^(2^NSQ) -> all rows == pi.  Squaring
     without transposes: keep (X, X^T); X2 = matmul(lhsT=X^T, rhs=X),
     X2^T = matmul(lhsT=X, rhs=X^T).  NSQ=3 -> P^8 (|lambda2(P)| ~ 0.38,
     so P^8 is within ~4e-4 of pi; gate is 2e-2).  All matmul operands are
     bf16 (fp32 PE matmuls run 2 passes; bf16 runs 1 -- host-emulated
     max rel err of the full bf16 chain is 2.6e-3, still 7x under the
     gate).  The bf16 identity for the first transpose is packed into the
     f32 input blob and read back via AP bitcast.  P^T comes from one
     matmul against it (lhsT=P -> P^T), off the activation engine.
  3. The final squaring is fused with the partition broadcast:
     row0(X@X) = (XT[:,0])^T @ X, so matmul(lhsT=XT[:,0] bcast to (4,128),
     rhs=X) yields a (128, 4) PSUM tile whose every row is pi.
  4. Output streaming (16 SDMA engines at their ~27 GB/s ceiling; packet
     cost measured on HW: 4KiB pkt ~164 ns, 16KiB pkt ~612 ns, so bigger
     descriptors win once the pattern is wide enough):
       - Copy pi into a tiny [128, 4] SBUF seed, fill a 4 KiB-per-partition
         narrow slice (DVE + idle ACT split), and immediately stream the
         first 1 MiB from it (4 KiB descriptors).
       - Widen the pattern to 12 KiB per partition (DVE), then stream
         6 MiB as one dma_start (12 KiB descriptors) and the last 1 MiB
         as a second (8 KiB descriptors), so the SDMA queues never drain
         and the small-descriptor region is only the first 1 MiB.
     The DRAM output is declared flat [2097152]; the bytes are one value
     broadcast, so any host reshape is valid.
  5. Residual variance: under cross-core HBM load one SDMA engine
     (usually 15) intermittently drops to ~60% rate for a few us and
     finishes ~2-3 us after the rest.  Per-engine byte skew was tried in
     a previous session and refuted: partition-sliced DMAs are re-split
     across engines by the AP normalizer's own policy, so a straggling
     engine cannot be de-loaded.  Accepted.
"""

import sys

if "/opt/trn_rl_repo" not in sys.path:
    sys.path.insert(0, "/opt/trn_rl_repo")

import numpy as np

A = 4
V = 512
S = 8192
N_CORES = 8
PER_CORE = V * S * A // N_CORES  # 2,097,152 f32 = 8 MiB
P128 = 128
FREE = 3072                      # full pattern width (12 KiB per partition)
W0 = 1024                        # first-chunk pattern width (4 KiB/partition)
WC = 2048                        # last-chunk descriptor width (8 KiB)
NSQ = 3                          # total squarings incl. the fused final one

_cache = {}


def _build():
    import concourse.bacc as bacc
    import concourse.mybir as mybir
    import concourse.tile as tile

    f32 = mybir.dt.float32
    bf16 = mybir.dt.bfloat16
    AF = mybir.ActivationFunctionType

    nc = bacc.Bacc(
        "TRN2", target_bir_lowering=False, debug=False, num_devices=N_CORES
    )
    blob = nc.dram_tensor("blob", [A, 2 * A], f32, kind="ExternalInput").ap()
    out = nc.dram_tensor("out", [PER_CORE], f32, kind="ExternalOutput").ap()

    with tile.TileContext(nc) as tc:
        with (
            tc.tile_pool(name="small", bufs=1) as sp,
            tc.tile_pool(name="loop", bufs=3) as lp,
            tc.tile_pool(name="patt", bufs=1) as pp,
            tc.tile_pool(name="ps1", bufs=1, space="PSUM") as ps1,
            tc.tile_pool(name="ps2", bufs=3, space="PSUM") as ps2,
        ):
            bt = sp.tile([A, 2 * A], f32)
            nc.sync.dma_start(out=bt[:], in_=blob, single_packet=True)
            lq = bt[:, 0:A]                 # logq, diagonal pre-masked to -100
            # cols [A : A+2] hold the bf16 identity's raw bits
            eye_bf = bt[:, A : A + 2].bitcast(bf16)   # [A, A] bf16

            E = sp.tile([A, A], f32)        # exp(lq): zero diagonal
            s = sp.tile([A, 1], f32)        # fused row sums
            nc.scalar.activation(out=E[:], in_=lq, func=AF.Exp, accum_out=s[:])
            r = sp.tile([A, 1], f32)
            nc.vector.reciprocal(out=r[:], in_=s[:])

            # X = P = diag(r) @ E, cast to bf16 for 1-pass PE matmuls
            X0 = sp.tile([A, A], bf16)
            nc.vector.tensor_scalar_mul(out=X0[:], in0=E[:], scalar1=r[:])
            # X^T via one matmul: lhsT=X0 -> X0^T @ I = P^T  (no PE transpose)
            pt = ps1.tile([A, A], f32)
            nc.tensor.matmul(pt[:], lhsT=X0[:], rhs=eye_bf, start=True, stop=True)
            XT0 = sp.tile([A, A], bf16)
            nc.vector.tensor_copy(out=XT0[:], in_=pt[:])

            # Squaring loop.  Both matmuls of an iteration write bank-aligned
            # quads of ONE two-bank PSUM tile, so a single strided DVE copy
            # (instead of two engine-split copies) pulls X2 and X2^T back to
            # SBUF side by side.
            BANK = 512  # f32 elems per PSUM bank row
            Xa, XTa = X0, XT0
            for _ in range(NSQ - 1):
                pr = ps2.tile([A, 2 * BANK], f32)
                nc.tensor.matmul(
                    pr[:, 0:A], lhsT=XTa[:], rhs=Xa[:], start=True, stop=True
                )
                nc.tensor.matmul(
                    pr[:, BANK : BANK + A], lhsT=Xa[:], rhs=XTa[:],
                    start=True, stop=True,
                )
                pair = lp.tile([A, 2 * A], bf16)
                psrc = pr[:].rearrange("p (b f) -> p b f", b=2)[:, :, 0:A]
                pdst = pair[:].rearrange("p (b f) -> p b f", b=2)
                nc.vector.tensor_copy(out=pdst, in_=psrc)
                Xa, XTa = pair[:, 0:A], pair[:, A : 2 * A]

            # Fused last squaring + broadcast:
            # row0(X@X) = (XT[:,0])^T @ X, replicated to 128 partitions by
            # free-dim-broadcasting the stationary operand.
            pbig = ps1.tile([P128, A], f32)
            nc.tensor.matmul(
                pbig[:],
                lhsT=XTa[:, 0:1].to_broadcast((A, P128)),
                rhs=Xa[:],
                start=True,
                stop=True,
            )

            # Stream out: narrow fill -> first chunk ASAP -> widen -> rest.
            # DVE broadcast-reads from PSUM run at ~1.2 ns/elem vs ~0.6 from
            # SBUF (measured), so hop pi through a tiny SBUF seed first.
            seed = sp.tile([P128, A], f32)
            nc.vector.tensor_copy(out=seed[:], in_=pbig[:])
            patt = pp.tile([P128, FREE], f32)
            # First-chunk fill split across DVE and the idle activation
            # engine (its Copy/scale funcs share the Exp table set, so no
            # extra ACT_TABLE_LOAD) to shave the critical path.
            WD = 768  # DVE's share; ACT fills the rest of W0
            p3a = patt[:, 0:WD].rearrange("p (r a) -> p r a", a=A)
            src_a = seed[:].unsqueeze(1).to_broadcast((P128, WD // A, A))
            nc.vector.tensor_copy(out=p3a, in_=src_a)
            p3c = patt[:, WD:W0].rearrange("p (r a) -> p r a", a=A)
            src_c = seed[:].unsqueeze(1).to_broadcast((P128, (W0 - WD) // A, A))
            nc.scalar.mul(p3c, src_c, 1.0)
            # A: first 1 MiB from the narrow slice, all 128 partitions.
            o0, l0 = 0, P128 * 2 * W0
            nc.sync.dma_start(
                out=out[o0 : o0 + l0].rearrange(
                    "(p c f) -> p c f", p=P128, f=W0
                ),
                in_=patt[:, 0:W0].unsqueeze(1).to_broadcast((P128, 2, W0)),
            )
            p3b = patt[:, W0:FREE].rearrange("p (r a) -> p r a", a=A)
            src_b = seed[:].unsqueeze(1).to_broadcast(
                (P128, (FREE - W0) // A, A)
            )
            nc.vector.tensor_copy(out=p3b, in_=src_b)
            # B: middle 6 MiB as one dma_start, 12 KiB descriptors.
            o1, l1 = o0 + l0, P128 * 4 * FREE
            nc.sync.dma_start(
                out=out[o1 : o1 + l1].rearrange(
                    "(p c f) -> p c f", p=P128, f=FREE
                ),
                in_=patt[:].unsqueeze(1).to_broadcast((P128, 4, FREE)),
            )
            # C: last 1 MiB, 8 KiB descriptors.
            o2, l2 = o1 + l1, P128 * WC
            nc.sync.dma_start(
                out=out[o2 : o2 + l2].rearrange("(p f) -> p f", p=P128),
                in_=patt[:, 0:WC],
            )
            assert o2 + l2 == PER_CORE

    nc.compile()
    return nc


def _get_nc():
    if "nc" not in _cache:
        _cache["nc"] = _build()
    return _cache["nc"]


def _in_map(log_Q_matrix_AxA):
    logq = np.asarray(log_Q_matrix_AxA, dtype=np.float32).reshape(A, A)
    eye = np.eye(A, dtype=np.float32)
    blob = np.zeros((A, 2 * A), dtype=np.float32)
    blob[:, 0:A] = logq - 100.0 * eye
    # bf16 identity bits: 1.0 = 0x3F80, packed pairs into f32 lanes
    eye_bits = np.where(np.eye(A, dtype=bool), 0x3F80, 0).astype("<u2")
    blob[:, A : A + 2] = eye_bits.view("<u4").view("<f4")
    return {"blob": np.ascontiguousarray(blob)}


def kernel(
    embeddings_VxD=None, site_positions_SxC=None, log_Q_matrix_AxA=None, **_unused
):
    from concourse.bass_utils import run_bass_kernel_spmd

    nc = _get_nc()
    im = _in_map(log_Q_matrix_AxA)
    res = run_bass_kernel_spmd(
        nc, [dict(im) for _ in range(N_CORES)], core_ids=list(range(N_CORES))
    )
    parts = [r["out"].reshape(V // N_CORES, S, A) for r in res.results]
    return np.concatenate(parts, axis=0)
